# revision 35
# baseline (speedup 1.0000x reference)
"""Trainium2 Bass kernel for DifferentiableNeuralGas loss (v2: pair-transposed).

loss = mean(exp(-(soft_rank-1)/LAMBDA) * distances) over [N, K]
  distances[n,k] = ||data[n] - weights[k]||_2
  soft_rank[n,i] = 1 + sum_{j != i} sigmoid((d[n,i]-d[n,j])/TAU)

Shift enumeration of the K*(K-1)/2 pair triangle: for shift s=1..64 the
128 ordered pairs (p, (p+s)%128) cover each unordered pair exactly once
(s=64 via the tanh antisymmetry).  With distances D[k, n] living
k-on-partitions (phase A output), the per-shift sigmoid arguments are
ARG_s = M_s^T @ D (M_s = I - P_s, a +/-1 selection matrix, one 128x128
fp16 matmul per 512 data columns), tanh'd on ACT, and the soft-rank sums
come back via a second matmul with the +/-1 matrix Mr_s -- no DVE fold
trees at all:

  rank-1 [g, n] = 63.5 + 0.5 * sum_s Mr_s^T @ tanh(ARG_s / (2*TAU))

accumulated directly in PSUM across all 64 shifts.  tanh values are
stored fp8e4m3 so pairs of shifts reduce in one DoubleRow matmul (2
k-tiles, 0.5 cycles/col); e4m3 tanh quantization costs ~6e-3 relative
loss error (vs the 2e-2 gate).  MODE="fp16" switches to exact fp16
reduce matmuls (2x PE cost).  NDVE shifts are produced on the otherwise
idle DVE instead of the PE: DMA partition-rotates D, one fp16 subtract,
tanh reads SBUF 2048-wide.

Tail: exp on ACT straight from S-PSUM (scale -1/(2*LAMBDA), bias
-63.5/LAMBDA), then one fused multiply+accumulate E*D on DVE ->
per-partition partials DMA'd out; host sums 8x128 values.
"""

import sys

sys.path.insert(0, "/opt/trn_rl_repo")

from contextlib import ExitStack

import numpy as np
import ml_dtypes

import concourse.bass as bass
import concourse.mybir as mybir
import concourse.tile as tile
from concourse import bacc
from concourse.bass_utils import run_bass_kernel_spmd


def _install_ntff_hook():
    """The agent image's antenv lacks axon_hooks, so trn_boot's NTFF
    profile hook never registers; recreate the tiny registry here so
    trace=True can capture HW profiles through libaxon_pjrt."""
    import types

    if "antenv.axon_hooks" in sys.modules:
        return
    mod = types.ModuleType("antenv.axon_hooks")
    _hook = [None]
    mod.set_axon_ntff_profile_hook = lambda h: _hook.__setitem__(0, h)
    mod.get_axon_ntff_profile_hook = lambda: _hook[0]
    sys.modules["antenv.axon_hooks"] = mod
    try:
        import trn_agent_boot.trn_boot as tb

        mod.set_axon_ntff_profile_hook(
            tb._ntff_profile_via_ctypes("/opt/axon/libaxon_pjrt.so"))
    except Exception:
        pass


_install_ntff_hook()

F32 = mybir.dt.float32
F16 = mybir.dt.float16
F8 = mybir.dt.float8e4
AF = mybir.ActivationFunctionType
ALU = mybir.AluOpType

N, D, K = 16384, 64, 128
NCORES = 8
TAU = 0.2
LAMBDA = 8.0
P = 128
NSH = 64             # shifts 1..64 cover the full pair triangle
SLAB = 1024          # data columns per ARG-psum buffer
MM = 512             # psum-bank-width matmul output cap (fp32)
MODE = "dr8"         # "dr8": fp8 tanh + DoubleRow reduce | "fp16": exact
NDVE = 60            # shifts whose ARG is built on DVE via rotated-D DMA
                     # (first (64-NDVE) shifts use PE matmuls: they need no
                     # DRAM round-trip, hiding the rot-pipeline fill at start)


def build(nloc: int) -> bass.Bass:
    nslab = nloc // SLAB
    nreg = nloc // MM
    t_dt = F8 if MODE == "dr8" else F16
    mr_dt = F8 if MODE == "dr8" else F16

    # shifts produced on DVE: pairs (2*pi+1, 2*pi+2) are all-DVE or
    # all-PE so each pair gets a single full-width tanh
    ndvepair = min(NDVE, NSH) // 2
    dve_shifts = set()
    for pi in range(NSH // 2 - ndvepair, NSH // 2):
        dve_shifts |= {2 * pi + 1, 2 * pi + 2}

    nc = bacc.Bacc()
    xT_d = nc.dram_tensor("xT", [D, nloc], F16, kind="ExternalInput")
    wTm2_d = nc.dram_tensor("wTm2", [D, K], F16, kind="ExternalInput")
    w2col_d = nc.dram_tensor("w2col", [K, 1], F32, kind="ExternalInput")
    M_d = nc.dram_tensor("M", [K, NSH * K], F16, kind="ExternalInput")
    Mr_d = nc.dram_tensor("Mr", [K, NSH * K], mr_dt, kind="ExternalInput")
    out_d = nc.dram_tensor("out", [1, 1], F32, kind="ExternalOutput")

    with ExitStack() as ctx:
        tc = ctx.enter_context(tile.TileContext(nc))
        singles = ctx.enter_context(tc.tile_pool(name="singles", bufs=1))

        # deadline-ordered sync-queue DMAs: xT block 0 unblocks phase A,
        # everything else follows in need order
        xT_all = singles.tile([D, nloc], F16, tag="xT_all")
        BB = min(512, nloc)
        nc.sync.dma_start(out=xT_all[:, 0:BB], in_=xT_d[:, 0:BB])
        wT_m2 = singles.tile([D, K], F16, tag="wTm2")
        nc.sync.dma_start(out=wT_m2, in_=wTm2_d[:, :])
        w2col = singles.tile([K, 1], F32, tag="w2col")
        nc.sync.dma_start(out=w2col, in_=w2col_d[:, :])
        npe_sh = NSH - len(dve_shifts)
        if npe_sh:
            M_sb = singles.tile([K, npe_sh * K], F16, tag="M")
            nc.sync.dma_start(out=M_sb, in_=M_d[:, 0:npe_sh * K])
        for b in range(1, nloc // BB):
            nc.sync.dma_start(out=xT_all[:, b * BB:(b + 1) * BB],
                              in_=xT_d[:, b * BB:(b + 1) * BB])
        Mr_sb = singles.tile([K, NSH * K], mr_dt, tag="Mr")
        for c in range(2):
            w = NSH * K // 2
            nc.sync.dma_start(out=Mr_sb[:, c * w:(c + 1) * w],
                              in_=Mr_d[:, c * w:(c + 1) * w])
        ones64 = singles.tile([D, P], F16, tag="ones64")
        nc.vector.memset(ones64, 1.0)
        onesP = singles.tile([P, 1], F32, tag="onesP")
        nc.vector.memset(onesP, 1.0)
        expbias = singles.tile([P, 1], F32, tag="expbias")
        nc.vector.memset(expbias, -63.5 / LAMBDA)

        # ---------------- phase A: distances (k on partitions) ----------
        D_all = singles.tile([K, nloc], F16, tag="D_all")
        with tc.tile_pool(name="psumA", bufs=2, space="PSUM") as psumA:
            # dependency-free warm-up matmuls fill the dead window while
            # the x tiles are in flight: PE hits its full 2.4GHz p-state
            # (3us continuous busy) right as phase A's real matmuls start
            for wu in range(24):
                warm0 = psumA.tile([P, P], F32, tag="warm", bufs=2)
                nc.tensor.matmul(warm0, ones64, ones64,
                                 start=True, stop=True,
                                 skip_group_check=True)
            xsq_all = singles.tile([D, nloc], F16, tag="xsq_all")
            for hh in range(nloc // (2 * BB)):
                psum_dT = psumA.tile([K, 2 * BB], F32, tag="dT")
                for j in range(2):
                    b = 2 * hh + j
                    sl = slice(b * BB, (b + 1) * BB)
                    ps = slice(j * BB, (j + 1) * BB)
                    nc.vector.scalar_tensor_tensor(
                        out=xsq_all[:, sl], in0=xT_all[:, sl], scalar=1.0,
                        in1=xT_all[:, sl], op0=ALU.bypass, op1=ALU.mult)
                    nc.tensor.matmul(psum_dT[:, ps], wT_m2, xT_all[:, sl],
                                     start=True, stop=False,
                                     skip_group_check=True)
                    nc.tensor.matmul(psum_dT[:, ps], ones64, xsq_all[:, sl],
                                     start=False, stop=True,
                                     skip_group_check=True)
                nc.scalar.activation(
                    D_all[:, hh * 2 * BB:(hh + 1) * 2 * BB], psum_dT,
                    AF.Sqrt, bias=w2col, scale=1.0)

        # doubled copy of D in DRAM: rot_s is then ONE contiguous-source
        # HWDGE load of rows [s, s+K) -- per-partition-descriptor
        # SBUF->SBUF shuffles are SWDGE-slow
        D2_dram = singles.tile([K + K // 2, nloc], F16, space="DRAM",
                               tag="D2dram")
        HB = nloc // 2
        nc.sync.dma_start(out=D2_dram[0:K, 0:HB], in_=D_all[:, 0:HB])
        nc.sync.dma_start(out=D2_dram[0:K, HB:nloc], in_=D_all[:, HB:nloc])
        nc.sync.dma_start(out=D2_dram[K:K + K // 2, :], in_=D_all[0:K // 2, :])

        # ---------------- phase B: tanh pair terms + PE reduce ----------
        with tc.tile_pool(name="psumB", bufs=1, space="PSUM") as psumB, \
             tc.tile_pool(name="work", bufs=1) as work:
            psumS = argp = psumB
            tp = rotp = argsp_pool = work
            S_ps = psumS.tile([P, nloc], F32, tag="S", bufs=1)
            npair = NSH // 2
            assert MODE == "dr8"

            def dr_reduce(pi, tbuf, off):
                """DoubleRow reduce of pair pi; its two shifts' tanh values
                sit at tbuf[:, off:off+nloc] and tbuf[:, off+nloc:off+2*nloc]."""
                lhs = Mr_sb[:, pi * 2 * K:(pi + 1) * 2 * K].rearrange(
                    "p (h g) -> p h g", h=2)
                tv = tbuf[:, off:off + 2 * nloc].rearrange(
                    "p (h e) -> p h e", h=2)
                for x in range(nreg):
                    nc.tensor.matmul(
                        S_ps[:, x * MM:(x + 1) * MM],
                        lhs, tv[:, :, x * MM:(x + 1) * MM],
                        start=(pi == 0), stop=(pi == npair - 1),
                        perf_mode=mybir.MatmulPerfMode.DoubleRow,
                        skip_group_check=True)

            # PE-ARG pairs (no DRAM round-trip: they cover the rot
            # pipeline fill at startup)
            npe_pair = npe_sh // 2
            pe_tpairs = []
            for pi in range(npe_pair):
                tpair = tp.tile([P, 2 * nloc], t_dt, tag="tpair", bufs=2)
                pe_tpairs.append(tpair)
                for h in range(2):
                    s = 2 * pi + h + 1
                    for q in range(nslab):
                        arg = argp.tile([P, SLAB], F32, tag="arg", bufs=2)
                        for m in range(SLAB // MM):
                            nc.tensor.matmul(
                                arg[:, m * MM:(m + 1) * MM],
                                M_sb[:, (s - 1) * K:s * K],
                                D_all[:, q * SLAB + m * MM:
                                      q * SLAB + (m + 1) * MM],
                                start=True, stop=True,
                                skip_group_check=True)
                        nc.scalar.activation(
                            tpair[:, h * nloc + q * SLAB:
                                  h * nloc + (q + 1) * SLAB],
                            arg, AF.Tanh, bias=0.0, scale=1.0 / (2 * TAU))
            for pi in range(npe_pair):
                dr_reduce(pi, pe_tpairs[pi], 0)

            # DVE chunks: rotated-D subtracts feeding one wide tanh per
            # chunk (amortizes ACT's ~352-cycle bubble).  Leading pairs
            # bridge the PE pairs while the store->rot pipeline fills;
            # trailing pairs keep the post-last-tanh DR tail short.
            sizes = [2, 2, 2] + [4] * ((NSH - npe_sh - 12) // 4) + [2, 2, 2]
            assert sum(sizes) == NSH - npe_sh
            s0 = npe_sh + 1
            pi0 = npe_pair
            for csz in sizes:
                tch = tp.tile([P, 4 * nloc], t_dt, tag="tquad", bufs=4)
                argsq = argsp_pool.tile([K, 4 * nloc], F16, tag="argsq",
                                        bufs=3)
                for h in range(csz):
                    s = s0 + h
                    rot = rotp.tile([K, nloc], F16, tag="rot", bufs=10)
                    nc.sync.dma_start(out=rot, in_=D2_dram[s:s + K, :])
                    nc.vector.tensor_tensor(
                        out=argsq[:, h * nloc:(h + 1) * nloc],
                        in0=D_all, in1=rot, op=ALU.subtract)
                nc.scalar.activation(tch[:, 0:csz * nloc],
                                     argsq[:, 0:csz * nloc], AF.Tanh,
                                     bias=0.0, scale=1.0 / (2 * TAU))
                if pi0 + csz // 2 == npair:
                    # keep the PE out of its low p-state during the last
                    # tanh so the exposed final reduces run ~1.5x faster;
                    # scratch matmuls on persistent tiles, free arg banks
                    for wu in range(5):
                        warm = argp.tile([P, SLAB], F32, tag="arg", bufs=2)
                        nc.tensor.matmul(
                            warm[:, 0:MM], Mr_sb[:, 0:K],
                            Mr_sb[:, 0:MM], start=True, stop=True,
                            skip_group_check=True)
                for hp in range(csz // 2):
                    dr_reduce(pi0, tch, hp * 2 * nloc)
                    pi0 += 1
                s0 += csz
            assert pi0 == npair and s0 == NSH + 1

            # ---------------- tail: exp + fused multiply-accumulate -----
            # halves overlap exp (ACT) with the multiply-accumulate (DVE);
            # the two per-partition partials collapse via fp32 ones-matmuls
            # accumulated in PSUM so the output DMA is ONE 4-byte
            # descriptor -- a [128,1] out DMA costs ~6us of semaphore
            # propagation.  S's banks are free once exp read them.
            E_all = singles.tile([P, nloc], F16, tag="E_all")
            scr = singles.tile([P, nloc], F16, tag="scr")
            S2 = psumS.tile([P, nloc], F32, tag="S", bufs=1)
            H = nloc // 2
            losstot0 = singles.tile([P, 1], F32, tag="losstot0")
            losstot1 = singles.tile([P, 1], F32, tag="losstot1")
            losstot = [losstot0, losstot1]
            for i in range(2):
                hs = slice(i * H, (i + 1) * H)
                nc.scalar.activation(E_all[:, hs], S_ps[:, hs], AF.Exp,
                                     bias=expbias,
                                     scale=-1.0 / (2 * LAMBDA))
                nc.vector.scalar_tensor_tensor(
                    out=scr[:, hs], in0=E_all[:, hs], scalar=1.0,
                    in1=D_all[:, hs], op0=ALU.bypass, op1=ALU.mult,
                    accum_out=losstot[i])
                nc.tensor.matmul(S2[0:1, 0:1], onesP, losstot[i],
                                 start=(i == 0), stop=(i == 1))
            outsb = singles.tile([1, 1], F32, tag="outsb")
            nc.vector.tensor_copy(outsb, S2[0:1, 0:1])
            nc.sync.dma_start(out=out_d[:, :], in_=outsb)

    nc.finalize()
    return nc


def make_M() -> np.ndarray:
    M = np.zeros((K, NSH * K), dtype=np.float32)
    for s in range(1, NSH + 1):
        for p in range(K):
            M[p, (s - 1) * K + p] += 1.0
            M[(p + s) % K, (s - 1) * K + p] -= 1.0
    return np.ascontiguousarray(M.astype(np.float16))


def make_Mr() -> np.ndarray:
    Mr = np.zeros((K, NSH * K), dtype=np.float32)
    for s in range(1, NSH + 1):
        for p in range(K):
            Mr[p, (s - 1) * K + p] += 1.0
            if s <= NSH - 1:
                Mr[p, (s - 1) * K + (p + s) % K] -= 1.0
    dt = ml_dtypes.float8_e4m3 if MODE == "dr8" else np.float16
    return np.ascontiguousarray(Mr.astype(dt))


_BUILT: dict[int, bass.Bass] = {}


def get_built(nloc: int) -> bass.Bass:
    if nloc not in _BUILT:
        _BUILT[nloc] = build(nloc)
    return _BUILT[nloc]


def make_in_maps(data: np.ndarray, weights: np.ndarray, ncores: int):
    nloc = data.shape[0] // ncores
    M = make_M()
    Mr = make_Mr()
    wTm2 = np.ascontiguousarray((-2.0 * weights.T).astype(np.float16))
    w64 = weights.astype(np.float64)
    w2col = np.ascontiguousarray(
        (w64 * w64).sum(axis=1).astype(np.float32).reshape(K, 1))
    return [
        {
            "xT": np.ascontiguousarray(
                data[c * nloc:(c + 1) * nloc].T.astype(np.float16)),
            "wTm2": wTm2,
            "w2col": w2col,
            "M": M,
            "Mr": Mr,
        }
        for c in range(ncores)
    ]


def run(data, weights, trace: bool = False):
    """Returns (loss, BassKernelResults)."""
    data = np.ascontiguousarray(np.asarray(data, dtype=np.float32))
    weights = np.ascontiguousarray(np.asarray(weights, dtype=np.float32))
    n, k = data.shape[0], weights.shape[0]
    nloc = n // NCORES
    nc = get_built(nloc)
    in_maps = make_in_maps(data, weights, NCORES)
    res = run_bass_kernel_spmd(nc, in_maps, list(range(NCORES)), trace=trace)
    total = sum(float(r["out"].sum(dtype=np.float64)) for r in res.results)
    loss = np.float32(total / (n * k))
    return loss, res


def kernel(data, weights):
    loss, _ = run(data, weights)
    return loss
